# revision 3
# baseline (speedup 1.0000x reference)
"""Trainium2 Bass kernel for a pre-LN multi-head attention block (v2).

Computes, for x of shape (4, 2048, 512):
    xn  = LayerNorm(x) * gamma + beta
    q/k/v = xn @ W{q,k,v}.T + b{q,k,v}          (8 heads, dk=64)
    attn  = softmax(q k^T / sqrt(dk)) @ v
    out   = attn @ Wo.T + bo

Sharding: 8 cores = (4 batches) x (2 query-halves); per-core outputs are
disjoint row blocks, host gather is concatenation.

Structure (evolved from the 304us baseline; measured 264us):
  - gamma/beta folded into Wq/Wk/Wv + biases and the V bias folded
    through softmax into bo (attention weights sum to 1), all on the
    host; the LayerNorm transpose evacuations run on the otherwise-idle
    ScalarE as plain copies.
  - The attention inner loop is paced by ScalarE exp (~1.29us per
    (head, key-tile) measured); the PE stream per tile is scores(2) +
    PV(2, software-pipelined one tile behind so head boundaries don't
    stall the exp chain) + one filler matmul, where fillers are the
    real Q/K projection matmuls for later heads (dummies only when
    those run out).  This keeps the PE ~97% dense, which matters
    because the PE_HAM activity monitor halves the PE clock whenever
    density drops and it then never recovers.
  - Softmax normalization: per-head reciprocal + gpsimd
    partition-broadcast + DVE multiply in 128-query chunks, deferred
    into the next head's loop; the last head's chunks pipeline against
    the dense y_proj matmul chains in the tail.
"""

import ml_dtypes
import numpy as np

import concourse.bass as bass
import concourse.mybir as mybir
import concourse.tile as tile
from concourse import bacc
from concourse.bass_utils import run_bass_kernel_spmd
from concourse.masks import make_identity

F32 = mybir.dt.float32
BF16 = mybir.dt.bfloat16
ALU = mybir.AluOpType
ACTF = mybir.ActivationFunctionType

P = 128          # partitions
DIM = 512        # model dim
H = 8            # heads
DK = 64          # head dim
NTOK = 2048      # tokens per core (one batch's sequence)
NQ = 1024        # queries per core (half the sequence)
CC = DIM // P    # 4 contraction chunks of 128
TT = NTOK // P   # 16 token tiles
JT = NTOK // P   # 16 key tiles
NB = 512         # moving-operand limit per matmul
EPS = 1e-5
SCALE = DK ** -0.5

N_CORES = 8
_BUILT = None


def _build():
    nc = bacc.Bacc("TRN2", target_bir_lowering=False, debug=False,
                   num_devices=N_CORES)

    xq = nc.dram_tensor("xq", [NTOK, DIM], F32, kind="ExternalInput")
    wqT = nc.dram_tensor("wqT", [DIM, DIM], BF16, kind="ExternalInput")
    wkT = nc.dram_tensor("wkT", [DIM, DIM], BF16, kind="ExternalInput")
    wvT = nc.dram_tensor("wvT", [DIM, DIM], BF16, kind="ExternalInput")
    woT = nc.dram_tensor("woT", [DK, H, DIM], BF16, kind="ExternalInput")
    qb_c = nc.dram_tensor("qb_c", [P, CC], F32, kind="ExternalInput")
    kb_c = nc.dram_tensor("kb_c", [P, CC], F32, kind="ExternalInput")
    bo_b = nc.dram_tensor("bo_b", [P, DIM], F32, kind="ExternalInput")
    y = nc.dram_tensor("y", [NQ, DIM], F32, kind="ExternalOutput")

    with tile.TileContext(nc) as tc:
        with (
            tc.tile_pool(name="const", bufs=1) as const,
            tc.tile_pool(name="persist", bufs=1) as persist,
            tc.tile_pool(name="lnp", bufs=6) as lnp,
            tc.tile_pool(name="stp", bufs=8) as stp,
            tc.tile_pool(name="epp", bufs=3) as epp,
            tc.tile_pool(name="otp", bufs=2) as otp,
            tc.tile_pool(name="rpp", bufs=4) as rpp,
            tc.tile_pool(name="outp", bufs=3) as outp,
            # PSUM: 4 banks (sps x2) + 2 banks (ops) + 2 banks (work) = 8
            tc.tile_pool(name="spp", bufs=2, space="PSUM") as spp,
            tc.tile_pool(name="opp", bufs=1, space="PSUM") as opp,
            tc.tile_pool(name="wpp", bufs=2, space="PSUM") as wpp,
        ):
            ident = const.tile([P, P], BF16)
            make_identity(nc, ident)
            wv = const.tile([P, CC, DIM], BF16)
            nc.sync.dma_start(out=wv, in_=wvT.ap().rearrange(
                "(cc p) d -> p cc d", p=P))
            xts = []
            for tt in range(TT):
                xt = lnp.tile([P, DIM], F32, tag="xt", name=f"xt{tt}")
                nc.sync.dma_start(out=xt, in_=xq.ap()[tt * P:(tt + 1) * P, :])
                xts.append(xt)
                if tt == 3:
                    wq = const.tile([P, CC, DIM], BF16)
                    nc.sync.dma_start(out=wq, in_=wqT.ap().rearrange(
                        "(cc p) d -> p cc d", p=P))
                    wk = const.tile([P, CC, DIM], BF16)
                    nc.sync.dma_start(out=wk, in_=wkT.ap().rearrange(
                        "(cc p) d -> p cc d", p=P))
                    qb = const.tile([P, CC], F32)
                    nc.sync.dma_start(out=qb, in_=qb_c.ap())
                    kb = const.tile([P, CC], F32)
                    nc.sync.dma_start(out=kb, in_=kb_c.ap())

            bob = const.tile([P, DIM], F32)
            nc.sync.dma_start(out=bob, in_=bo_b.ap())
            epst = const.tile([P, 1], F32)
            nc.vector.memset(epst, EPS)
            wo = const.tile([DK, H, DIM], BF16)
            nc.sync.dma_start(out=wo, in_=woT.ap())

            # Persistent activations.
            xnT = persist.tile([P, CC, NTOK], BF16)    # xn^T
            qt = persist.tile([P, CC, NQ], BF16)       # Q^T
            kt = persist.tile([P, CC, NTOK], BF16)     # K^T
            vp = persist.tile([P, JT, H, DK + 2], BF16)  # [V_h | 1 | 0]
            onT = persist.tile([DK, H, NQ], BF16)      # normalized O^T

            nc.vector.memset(vp[:, :, :, DK], 1.0)
            nc.vector.memset(vp[:, :, :, DK + 1], 0.0)

            # ---- projection helpers -------------------------------------
            def v_proj(j):
                ps = wpp.tile([P, DIM], F32, tag="w", name=f"v{j}")
                for cc in range(CC):
                    nc.tensor.matmul(ps, lhsT=xnT[:, cc, j * P:(j + 1) * P],
                                     rhs=wv[:, cc, :],
                                     start=(cc == 0), stop=(cc == CC - 1))
                nc.scalar.activation(
                    out=vp[:, j, :, 0:DK],
                    in_=ps.rearrange("p (h d) -> p h d", d=DK),
                    func=ACTF.Copy)

            def qk_chunk(w, bias, dst, t, ib, n=NB):
                # one (128, n) psum chunk of the Q^T or K^T projection
                ps = wpp.tile([P, NB], F32, tag="w", name=f"qk{t}_{ib}_{w.name}")
                for cc in range(CC):
                    nc.tensor.matmul(ps[:, 0:n], lhsT=w[:, cc, t * P:(t + 1) * P],
                                     rhs=xnT[:, cc, ib * NB:ib * NB + n],
                                     start=(cc == 0), stop=(cc == CC - 1))
                nc.vector.tensor_scalar(
                    out=dst[:, t, ib * NB:ib * NB + n], in0=ps[:, 0:n],
                    scalar1=bias[:, t:t + 1], scalar2=None, op0=ALU.add)

            # single-matmul filler units; the psum tile is allocated at cc=0
            # and held across the 4 cc pieces.  Pieces of one chunk are
            # always adjacent in the filler queue and nothing else
            # allocates from the "w" tag while a chunk is in flight, so
            # the pool's buffer rotation cannot reclaim it mid-chunk.
            chunk_ps = {}

            def qk_piece(w, bias, dst, t, ib, cc):
                key = (w.name, t, ib)
                if cc == 0:
                    chunk_ps[key] = wpp.tile([P, NB], F32, tag="w",
                                             name=f"qk{t}_{ib}_{w.name}")
                ps = chunk_ps[key]
                nc.tensor.matmul(ps, lhsT=w[:, cc, t * P:(t + 1) * P],
                                 rhs=xnT[:, cc, ib * NB:(ib + 1) * NB],
                                 start=(cc == 0), stop=(cc == CC - 1))
                if cc == CC - 1:
                    del chunk_ps[key]
                    nc.vector.tensor_scalar(
                        out=dst[:, t, ib * NB:(ib + 1) * NB], in0=ps,
                        scalar1=bias[:, t:t + 1], scalar2=None, op0=ALU.add)

            dummy_n = [0]

            def dummy(n=NB):
                ps = wpp.tile([P, 512], F32, tag="w", name=f"d{dummy_n[0]}")
                dummy_n[0] += 1
                nc.tensor.matmul(ps[:, 0:n], lhsT=wv[:, 0, 0:P],
                                 rhs=wv[:, 0, 0:n], start=True, stop=True)

            # ---- LayerNorm + transpose + V + first Q/K chunk ------------
            G = 4  # stats group: batch the sqrt+reciprocal across 4 tiles
            mvg = [persist.tile([P, G, 2], F32, name=f"mvg{g}")
                   for g in range(TT // G)]
            rsg = [persist.tile([P, G], F32, name=f"rsg{g}")
                   for g in range(TT // G)]

            def ln_stats(tt):
                g, gi = divmod(tt, G)
                stats = stp.tile([P, 6], F32)
                nc.vector.bn_stats(out=stats, in_=xts[tt])
                nc.vector.bn_aggr(out=mvg[g][:, gi, :], in_=stats)
                if gi == G - 1:
                    nc.scalar.activation(out=rsg[g], in_=mvg[g][:, :, 1],
                                         func=ACTF.Sqrt, bias=epst)
                    nc.vector.reciprocal(out=rsg[g], in_=rsg[g])

            for tt in range(G):
                ln_stats(tt)
            for tt in range(TT):
                if tt + G < TT:
                    ln_stats(tt + G)
                g, gi = divmod(tt, G)
                z = lnp.tile([P, DIM], BF16, tag="z")
                nc.vector.tensor_scalar(out=z, in0=xts[tt],
                                        scalar1=mvg[g][:, gi, 0:1],
                                        scalar2=rsg[g][:, gi:gi + 1],
                                        op0=ALU.subtract, op1=ALU.mult)
                zt4 = wpp.tile([P, DIM], BF16, tag="w", name=f"zt{tt}")
                for cc in range(CC):
                    nc.tensor.transpose(zt4[:, cc * P:(cc + 1) * P],
                                        z[:, cc * P:(cc + 1) * P], ident)
                nc.scalar.activation(
                    out=xnT[:, :, tt * P:(tt + 1) * P],
                    in_=zt4.rearrange("p (cc q) -> p cc q", cc=CC),
                    func=ACTF.Copy)
                v_proj(tt)
                if tt == 3:
                    qk_chunk(wq, qb, qt, 0, 0)
                    qk_chunk(wk, kb, kt, 0, 0)
                elif tt == 7:
                    qk_chunk(wq, qb, qt, 0, 1)
                    qk_chunk(wk, kb, kt, 0, 1)
                elif tt == 11:
                    qk_chunk(wk, kb, kt, 0, 2)
                elif tt == 15:
                    qk_chunk(wk, kb, kt, 0, 3)

            # ---- filler inventory: later-chunk Q/K projections ----------
            fillers = []
            for t in range(1, CC):
                for ib in range(NQ // NB):
                    for cc in range(CC):
                        fillers.append(
                            lambda t=t, ib=ib, cc=cc: qk_piece(wq, qb, qt, t, ib, cc))
                for ib in range(NTOK // NB):
                    for cc in range(CC):
                        fillers.append(
                            lambda t=t, ib=ib, cc=cc: qk_piece(wk, kb, kt, t, ib, cc))

            # ---- attention: software-pipelined, PV one tile behind ------
            # Emitting the PV pair for tile j AFTER the scores+exp of tile
            # j+1 lets the next exp's scores run while PV waits on this
            # exp (PE wait-queue), removing the head-boundary stalls.
            deferred = []          # pending norm-chunk thunks (prev heads)
            pend = None            # (h, j, et) awaiting its PV
            ops_by_head = {}

            def get_ops(h):
                if h not in ops_by_head:
                    ops_by_head[h] = opp.tile([DK + 2, NQ], F32, tag="ops",
                                              name=f"o{h}")
                return ops_by_head[h]

            def emit_pv(p):
                h, j, et = p
                ops = get_ops(h)
                for ib in range(NQ // NB):
                    nc.tensor.matmul(
                        ops[:, ib * NB:(ib + 1) * NB],
                        lhsT=vp[:, j, h, :],
                        rhs=et[:, ib * NB:(ib + 1) * NB],
                        start=(j == 0), stop=(j == JT - 1))

            def drain_head(h):
                # Drain the PSUM accumulator fast, then normalize from the
                # SBUF copy: reciprocal + partition-broadcast + multiply in
                # 128-query chunks (shapes kept small: proven-safe ucode).
                ot = otp.tile([DK + 1, NQ], F32, tag="ot", name=f"ot{h}")
                nc.vector.tensor_copy(out=ot, in_=ops_by_head[h][0:DK + 1, :])
                rinv = rpp.tile([1, NQ], F32, tag="r", name=f"r{h}")
                rb = rpp.tile([DK, NQ], F32, tag="rb", name=f"rb{h}")

                def recip_c(i):
                    cs = slice(i * P, (i + 1) * P)
                    nc.vector.reciprocal(out=rinv[:, cs], in_=ot[DK:DK + 1, cs])

                def bcast_c(i):
                    cs = slice(i * P, (i + 1) * P)
                    nc.gpsimd.partition_broadcast(rb[:, cs], rinv[:, cs])

                def mult_c(i, h=h):
                    cs = slice(i * P, (i + 1) * P)
                    nc.vector.tensor_tensor(out=onT[:, h, cs],
                                            in0=ot[0:DK, cs], in1=rb[:, cs],
                                            op=ALU.mult)
                return recip_c, bcast_c, mult_c

            for h in range(H):
                hp, hm = divmod(h, 2)
                kt_h = kt[hm * DK:(hm + 1) * DK, hp, :]
                qt_h = qt[hm * DK:(hm + 1) * DK, hp, :]
                for j in range(JT):
                    sps = spp.tile([P, NQ], F32, tag="sps", name=f"s{h}{j}")
                    for ib in range(NQ // NB):
                        nc.tensor.matmul(
                            sps[:, ib * NB:(ib + 1) * NB],
                            lhsT=kt_h[:, j * P:(j + 1) * P],
                            rhs=qt_h[:, ib * NB:(ib + 1) * NB],
                            start=True, stop=True)
                    et = epp.tile([P, NQ], BF16, tag="et", name=f"e{h}{j}")
                    nc.scalar.activation(out=et, in_=sps, func=ACTF.Exp,
                                         scale=SCALE)
                    if pend is not None:
                        emit_pv(pend)
                        if pend[1] == JT - 1:   # previous head complete
                            rc, bc_, mc = drain_head(pend[0])
                            deferred.extend(
                                lambda i=i, rc=rc, bc_=bc_, mc=mc:
                                (rc(i), bc_(i), mc(i))
                                for i in range(NQ // P))
                    pend = (h, j, et)
                    if fillers:
                        fillers.pop(0)()
                    else:
                        dummy()
                    if deferred and j >= 1:
                        deferred.pop(0)()

            # ---- tail: final PV, then normalization pipelined with dense
            # y_proj matmul chains (PE kept busy through the norm lead-in
            # so the activity monitor never halves the clock).
            emit_pv(pend)
            rc, bc_, mc = drain_head(H - 1)
            rc(0)
            rc(1)
            bc_(0)
            mc(0)
            for _ in range(16):
                dummy()

            def y_proj(it):
                yps = wpp.tile([P, DIM], F32, tag="w", name=f"y{it}")
                for h in range(H):
                    nc.tensor.matmul(
                        yps, lhsT=onT[:, h, it * P:(it + 1) * P],
                        rhs=wo[:, h, :],
                        start=(h == 0), stop=(h == H - 1))
                yo = outp.tile([P, DIM], F32)
                nc.vector.tensor_tensor(out=yo, in0=yps, in1=bob, op=ALU.add)
                nc.sync.dma_start(out=y.ap()[it * P:(it + 1) * P, :], in_=yo)

            for it in range(NQ // P):
                y_proj(it)
                if it + 2 < NQ // P:
                    rc(it + 2)
                if it + 1 < NQ // P:
                    bc_(it + 1)
                    mc(it + 1)

    nc.compile()
    return nc


def _get_nc():
    global _BUILT
    if _BUILT is None:
        _BUILT = _build()
    return _BUILT


def prep_in_maps(inputs):
    x = np.asarray(inputs["x"], np.float32)
    B, N, D = x.shape
    assert (B, N, D) == (4, 2048, 512)

    gam = np.asarray(inputs["ln_gamma"], np.float64)
    bet = np.asarray(inputs["ln_beta"], np.float64)
    Wq = np.asarray(inputs["Wq"], np.float64)
    Wk = np.asarray(inputs["Wk"], np.float64)
    Wv = np.asarray(inputs["Wv"], np.float64)

    # fold gamma/beta:  (xn*gam+bet) @ W.T + b  ==  xn @ (W*gam).T + (b + W@bet)
    Wo = np.asarray(inputs["Wo"], np.float64)
    bq = np.asarray(inputs["bq"], np.float64) + Wq @ bet
    bk = np.asarray(inputs["bk"], np.float64) + Wk @ bet
    bv = np.asarray(inputs["bv"], np.float64) + Wv @ bet
    bo = np.asarray(inputs["bo"], np.float64) + Wo @ bv
    Wqg = Wq * gam[None, :]
    Wkg = Wk * gam[None, :]
    Wvg = Wv * gam[None, :]

    def cols(v):  # (512,) -> (128, 4): column t = v[128t:128(t+1)]
        return np.ascontiguousarray(
            np.asarray(v, np.float32).reshape(CC, P).T)

    def bcast(v):  # (512,) -> (128, 512)
        return np.ascontiguousarray(
            np.broadcast_to(np.asarray(v, np.float32), (P, DIM)))

    bf16 = ml_dtypes.bfloat16
    common = {
        "wqT": np.ascontiguousarray(Wqg.astype(np.float32).T.astype(bf16)),
        "wkT": np.ascontiguousarray(Wkg.astype(np.float32).T.astype(bf16)),
        "wvT": np.ascontiguousarray(Wvg.astype(np.float32).T.astype(bf16)),
        "woT": np.ascontiguousarray(
            np.asarray(inputs["Wo"], np.float32).T
            .reshape(H, DK, DIM).transpose(1, 0, 2).astype(bf16)),
        "qb_c": cols(bq), "kb_c": cols(bk),
        "bo_b": bcast(bo),
    }
    in_maps = []
    for c in range(N_CORES):
        b, half = divmod(c, 2)
        o = half * NQ
        xc = np.concatenate([x[b, o:o + NQ], x[b, NQ - o:N - o]], axis=0)
        in_maps.append({"xq": np.ascontiguousarray(xc), **common})
    return in_maps


def kernel(x, ln_gamma, ln_beta, Wq, bq, Wk, bk, Wv, bv, Wo, bo):
    in_maps = prep_in_maps(dict(
        x=x, ln_gamma=ln_gamma, ln_beta=ln_beta, Wq=Wq, bq=bq, Wk=Wk, bk=bk,
        Wv=Wv, bv=bv, Wo=Wo, bo=bo))

    nc = _get_nc()
    res = run_bass_kernel_spmd(nc, in_maps, core_ids=list(range(N_CORES)))

    B, N, D = 4, 2048, DIM
    out = np.empty((B, N, D), np.float32)
    for c in range(N_CORES):
        b, half = divmod(c, 2)
        o = half * NQ
        out[b, o:o + NQ] = res.results[c]["y"]
    return out


# revision 5
# speedup vs baseline: 1.1389x; 1.1389x over previous
"""Trainium2 Bass kernel for a pre-LN multi-head attention block (v2).

Computes, for x of shape (4, 2048, 512):
    xn  = LayerNorm(x) * gamma + beta
    q/k/v = xn @ W{q,k,v}.T + b{q,k,v}          (8 heads, dk=64)
    attn  = softmax(q k^T / sqrt(dk)) @ v
    out   = attn @ Wo.T + bo

Sharding: 8 cores = (4 batches) x (2 query-halves); per-core outputs are
disjoint row blocks, host gather is concatenation.

Key structure (evolved from the 304us baseline; ~250-265us fast-state):
  - gamma/beta folded into Wq/Wk/Wv + biases, and the V bias folded
    through softmax into bo (attention weights sum to 1), all on the
    host; LayerNorm transpose evacuations are plain copies on the
    otherwise-idle ScalarE, rstd sqrt+reciprocal batched 4 tiles at a
    time, and V/QK projection emission lags its tile by one step so the
    PE never waits inline on the same tile's ACT evacuation.
  - The attention inner loop is paced by ScalarE exp (~1.29us per
    (head, key-tile) measured); the PE stream per tile is scores(2) +
    PV(2, software-pipelined one tile behind so head boundaries do not
    stall the exp chain) + one filler matmul, where fillers are the
    real Q/K projection matmuls for later heads (dummies only when
    those run out).  This keeps the PE ~97% dense, which matters
    because the PE_HAM activity monitor halves the PE clock whenever
    density drops and it then rarely recovers.
  - Softmax normalization: per-head reciprocal + gpsimd
    partition-broadcast + DVE multiply in 128-query chunks, deferred
    into the next head's loop.  The LAST head's chain is emitted in
    full before the first y_proj: any DVE work queued after the y
    bias-adds would head-of-line block them, and the psum-pool WAR on
    the 2-bank work ring then stalls later y chains long enough to
    trip the activity monitor into half-clock (this was 33-42us of
    half-clock tail before the fix).
"""

import ml_dtypes
import numpy as np

import concourse.bass as bass
import concourse.mybir as mybir
import concourse.tile as tile
from concourse import bacc
from concourse.bass_utils import run_bass_kernel_spmd
from concourse.masks import make_identity

F32 = mybir.dt.float32
BF16 = mybir.dt.bfloat16
ALU = mybir.AluOpType
ACTF = mybir.ActivationFunctionType

P = 128          # partitions
DIM = 512        # model dim
H = 8            # heads
DK = 64          # head dim
NTOK = 2048      # tokens per core (one batch's sequence)
NQ = 1024        # queries per core (half the sequence)
CC = DIM // P    # 4 contraction chunks of 128
TT = NTOK // P   # 16 token tiles
JT = NTOK // P   # 16 key tiles
NB = 512         # moving-operand limit per matmul
EPS = 1e-5
SCALE = DK ** -0.5

N_CORES = 8
_BUILT = None


def _build():
    nc = bacc.Bacc("TRN2", target_bir_lowering=False, debug=False,
                   num_devices=N_CORES)

    xq = nc.dram_tensor("xq", [NTOK, DIM], F32, kind="ExternalInput")
    wqT = nc.dram_tensor("wqT", [DIM, DIM], BF16, kind="ExternalInput")
    wkT = nc.dram_tensor("wkT", [DIM, DIM], BF16, kind="ExternalInput")
    wvT = nc.dram_tensor("wvT", [DIM, DIM], BF16, kind="ExternalInput")
    woT = nc.dram_tensor("woT", [DK, H, DIM], BF16, kind="ExternalInput")
    qb_c = nc.dram_tensor("qb_c", [P, CC], F32, kind="ExternalInput")
    kb_c = nc.dram_tensor("kb_c", [P, CC], F32, kind="ExternalInput")
    bo_b = nc.dram_tensor("bo_b", [P, DIM], F32, kind="ExternalInput")
    y = nc.dram_tensor("y", [NQ, DIM], F32, kind="ExternalOutput")

    with tile.TileContext(nc) as tc:
        with (
            tc.tile_pool(name="const", bufs=1) as const,
            tc.tile_pool(name="persist", bufs=1) as persist,
            tc.tile_pool(name="lnp", bufs=6) as lnp,
            tc.tile_pool(name="stp", bufs=8) as stp,
            tc.tile_pool(name="epp", bufs=3) as epp,
            tc.tile_pool(name="otp", bufs=2) as otp,
            tc.tile_pool(name="rpp", bufs=4) as rpp,
            tc.tile_pool(name="outp", bufs=3) as outp,
            # PSUM: 4 banks (sps x2) + 2 banks (ops) + 2 banks (work) = 8
            tc.tile_pool(name="spp", bufs=2, space="PSUM") as spp,
            tc.tile_pool(name="opp", bufs=1, space="PSUM") as opp,
            tc.tile_pool(name="wpp", bufs=2, space="PSUM") as wpp,
        ):
            ident = const.tile([P, P], BF16)
            make_identity(nc, ident)
            wv = const.tile([P, CC, DIM], BF16)
            nc.sync.dma_start(out=wv, in_=wvT.ap().rearrange(
                "(cc p) d -> p cc d", p=P))
            xts = []
            for tt in range(TT):
                xt = lnp.tile([P, DIM], F32, tag="xt", name=f"xt{tt}")
                nc.sync.dma_start(out=xt, in_=xq.ap()[tt * P:(tt + 1) * P, :])
                xts.append(xt)
                if tt == 3:
                    wq = const.tile([P, CC, DIM], BF16)
                    nc.sync.dma_start(out=wq, in_=wqT.ap().rearrange(
                        "(cc p) d -> p cc d", p=P))
                    wk = const.tile([P, CC, DIM], BF16)
                    nc.sync.dma_start(out=wk, in_=wkT.ap().rearrange(
                        "(cc p) d -> p cc d", p=P))
                    qb = const.tile([P, CC], F32)
                    nc.sync.dma_start(out=qb, in_=qb_c.ap())
                    kb = const.tile([P, CC], F32)
                    nc.sync.dma_start(out=kb, in_=kb_c.ap())

            bob = const.tile([P, DIM], F32)
            nc.sync.dma_start(out=bob, in_=bo_b.ap())
            epst = const.tile([P, 1], F32)
            nc.vector.memset(epst, EPS)
            wo = const.tile([DK, H, DIM], BF16)
            nc.sync.dma_start(out=wo, in_=woT.ap())

            # Persistent activations.
            xnT = persist.tile([P, CC, NTOK], BF16)    # xn^T
            qt = persist.tile([P, CC, NQ], BF16)       # Q^T
            kt = persist.tile([P, CC, NTOK], BF16)     # K^T
            vp = persist.tile([P, JT, H, DK + 2], BF16)  # [V_h | 1 | 0]
            onT = persist.tile([DK, H, NQ], BF16)      # normalized O^T

            nc.vector.memset(vp[:, :, :, DK], 1.0)
            nc.vector.memset(vp[:, :, :, DK + 1], 0.0)

            # ---- projection helpers -------------------------------------
            def v_proj(j):
                ps = wpp.tile([P, DIM], F32, tag="w", name=f"v{j}")
                for cc in range(CC):
                    nc.tensor.matmul(ps, lhsT=xnT[:, cc, j * P:(j + 1) * P],
                                     rhs=wv[:, cc, :],
                                     start=(cc == 0), stop=(cc == CC - 1))
                nc.scalar.activation(
                    out=vp[:, j, :, 0:DK],
                    in_=ps.rearrange("p (h d) -> p h d", d=DK),
                    func=ACTF.Copy)

            def qk_chunk(w, bias, dst, t, ib, n=NB):
                # one (128, n) psum chunk of the Q^T or K^T projection
                ps = wpp.tile([P, NB], F32, tag="w", name=f"qk{t}_{ib}_{w.name}")
                for cc in range(CC):
                    nc.tensor.matmul(ps[:, 0:n], lhsT=w[:, cc, t * P:(t + 1) * P],
                                     rhs=xnT[:, cc, ib * NB:ib * NB + n],
                                     start=(cc == 0), stop=(cc == CC - 1))
                nc.vector.tensor_scalar(
                    out=dst[:, t, ib * NB:ib * NB + n], in0=ps[:, 0:n],
                    scalar1=bias[:, t:t + 1], scalar2=None, op0=ALU.add)

            # single-matmul filler units; the psum tile is allocated at cc=0
            # and held across the 4 cc pieces.  Pieces of one chunk are
            # always adjacent in the filler queue and nothing else
            # allocates from the "w" tag while a chunk is in flight, so
            # the pool's buffer rotation cannot reclaim it mid-chunk.
            chunk_ps = {}

            def qk_piece(w, bias, dst, t, ib, cc):
                key = (w.name, t, ib)
                if cc == 0:
                    chunk_ps[key] = wpp.tile([P, NB], F32, tag="w",
                                             name=f"qk{t}_{ib}_{w.name}")
                ps = chunk_ps[key]
                nc.tensor.matmul(ps, lhsT=w[:, cc, t * P:(t + 1) * P],
                                 rhs=xnT[:, cc, ib * NB:(ib + 1) * NB],
                                 start=(cc == 0), stop=(cc == CC - 1))
                if cc == CC - 1:
                    del chunk_ps[key]
                    nc.vector.tensor_scalar(
                        out=dst[:, t, ib * NB:(ib + 1) * NB], in0=ps,
                        scalar1=bias[:, t:t + 1], scalar2=None, op0=ALU.add)

            dummy_n = [0]

            def dummy(n=NB):
                ps = wpp.tile([P, 512], F32, tag="w", name=f"d{dummy_n[0]}")
                dummy_n[0] += 1
                nc.tensor.matmul(ps[:, 0:n], lhsT=wv[:, 0, 0:P],
                                 rhs=wv[:, 0, 0:n], start=True, stop=True)

            # ---- LayerNorm + transpose + V + first Q/K chunk ------------
            G = 4  # stats group: batch the sqrt+reciprocal across 4 tiles
            mvg = [persist.tile([P, G, 2], F32, name=f"mvg{g}")
                   for g in range(TT // G)]
            rsg = [persist.tile([P, G], F32, name=f"rsg{g}")
                   for g in range(TT // G)]

            def ln_stats(tt):
                g, gi = divmod(tt, G)
                stats = stp.tile([P, 6], F32)
                nc.vector.bn_stats(out=stats, in_=xts[tt])
                nc.vector.bn_aggr(out=mvg[g][:, gi, :], in_=stats)
                if gi == G - 1:
                    nc.scalar.activation(out=rsg[g], in_=mvg[g][:, :, 1],
                                         func=ACTF.Sqrt, bias=epst)
                    nc.vector.reciprocal(out=rsg[g], in_=rsg[g])

            for tt in range(G):
                ln_stats(tt)
            for tt in range(TT):
                if tt + G < TT:
                    ln_stats(tt + G)
                g, gi = divmod(tt, G)
                z = lnp.tile([P, DIM], BF16, tag="z")
                nc.vector.tensor_scalar(out=z, in0=xts[tt],
                                        scalar1=mvg[g][:, gi, 0:1],
                                        scalar2=rsg[g][:, gi:gi + 1],
                                        op0=ALU.subtract, op1=ALU.mult)
                zt4 = wpp.tile([P, DIM], BF16, tag="w", name=f"zt{tt}")
                for cc in range(CC):
                    nc.tensor.transpose(zt4[:, cc * P:(cc + 1) * P],
                                        z[:, cc * P:(cc + 1) * P], ident)
                nc.scalar.activation(
                    out=xnT[:, :, tt * P:(tt + 1) * P],
                    in_=zt4.rearrange("p (cc q) -> p cc q", cc=CC),
                    func=ACTF.Copy)
                if tt >= 1:
                    v_proj(tt - 1)
                if tt == 4:
                    qk_chunk(wq, qb, qt, 0, 0)
                    qk_chunk(wk, kb, kt, 0, 0)
                elif tt == 8:
                    qk_chunk(wq, qb, qt, 0, 1)
                    qk_chunk(wk, kb, kt, 0, 1)
                elif tt == 12:
                    qk_chunk(wk, kb, kt, 0, 2)
            v_proj(TT - 1)
            qk_chunk(wk, kb, kt, 0, 3)

            # ---- filler inventory: later-chunk Q/K projections ----------
            fillers = []
            for t in range(1, CC):
                for ib in range(NQ // NB):
                    for cc in range(CC):
                        fillers.append(
                            lambda t=t, ib=ib, cc=cc: qk_piece(wq, qb, qt, t, ib, cc))
                for ib in range(NTOK // NB):
                    for cc in range(CC):
                        fillers.append(
                            lambda t=t, ib=ib, cc=cc: qk_piece(wk, kb, kt, t, ib, cc))

            # ---- attention: software-pipelined, PV one tile behind ------
            # Emitting the PV pair for tile j AFTER the scores+exp of tile
            # j+1 lets the next exp's scores run while PV waits on this
            # exp (PE wait-queue), removing the head-boundary stalls.
            deferred = []          # pending norm-chunk thunks (prev heads)
            pend = None            # (h, j, et) awaiting its PV
            ops_by_head = {}

            def get_ops(h):
                if h not in ops_by_head:
                    ops_by_head[h] = opp.tile([DK + 2, NQ], F32, tag="ops",
                                              name=f"o{h}")
                return ops_by_head[h]

            def emit_pv(p):
                h, j, et = p
                ops = get_ops(h)
                for ib in range(NQ // NB):
                    nc.tensor.matmul(
                        ops[:, ib * NB:(ib + 1) * NB],
                        lhsT=vp[:, j, h, :],
                        rhs=et[:, ib * NB:(ib + 1) * NB],
                        start=(j == 0), stop=(j == JT - 1))

            def drain_head(h):
                # Drain the PSUM accumulator fast, then normalize from the
                # SBUF copy: reciprocal + partition-broadcast + multiply in
                # 128-query chunks (shapes kept small: proven-safe ucode).
                ot = otp.tile([DK + 1, NQ], F32, tag="ot", name=f"ot{h}")
                nc.vector.tensor_copy(out=ot, in_=ops_by_head[h][0:DK + 1, :])
                rinv = rpp.tile([1, NQ], F32, tag="r", name=f"r{h}")
                rb = rpp.tile([DK, NQ], F32, tag="rb", name=f"rb{h}")

                def recip_c(i):
                    cs = slice(i * P, (i + 1) * P)
                    nc.vector.reciprocal(out=rinv[:, cs], in_=ot[DK:DK + 1, cs])

                def bcast_c(i):
                    cs = slice(i * P, (i + 1) * P)
                    nc.gpsimd.partition_broadcast(rb[:, cs], rinv[:, cs])

                def mult_c(i, h=h):
                    cs = slice(i * P, (i + 1) * P)
                    nc.vector.tensor_tensor(out=onT[:, h, cs],
                                            in0=ot[0:DK, cs], in1=rb[:, cs],
                                            op=ALU.mult)
                return recip_c, bcast_c, mult_c

            for h in range(H):
                hp, hm = divmod(h, 2)
                kt_h = kt[hm * DK:(hm + 1) * DK, hp, :]
                qt_h = qt[hm * DK:(hm + 1) * DK, hp, :]
                for j in range(JT):
                    sps = spp.tile([P, NQ], F32, tag="sps", name=f"s{h}{j}")
                    for ib in range(NQ // NB):
                        nc.tensor.matmul(
                            sps[:, ib * NB:(ib + 1) * NB],
                            lhsT=kt_h[:, j * P:(j + 1) * P],
                            rhs=qt_h[:, ib * NB:(ib + 1) * NB],
                            start=True, stop=True)
                    et = epp.tile([P, NQ], BF16, tag="et", name=f"e{h}{j}")
                    nc.scalar.activation(out=et, in_=sps, func=ACTF.Exp,
                                         scale=SCALE)
                    if pend is not None:
                        emit_pv(pend)
                        if pend[1] == JT - 1:   # previous head complete
                            rc, bc_, mc = drain_head(pend[0])
                            deferred.extend(
                                lambda i=i, rc=rc, bc_=bc_, mc=mc:
                                (rc(i), bc_(i), mc(i))
                                for i in range(NQ // P))
                    pend = (h, j, et)
                    if fillers:
                        fillers.pop(0)()
                    else:
                        dummy()
                    if deferred and j >= 1:
                        deferred.pop(0)()

            # ---- tail: final PV, then normalization pipelined with dense
            # y_proj matmul chains (PE kept busy through the norm lead-in
            # so the activity monitor never halves the clock).
            # Emit the ENTIRE last-head norm chain before the first y_proj:
            # any DVE work queued after the y bias-adds would head-of-line
            # block them, and the psum-pool WAR (y reuses the 2-bank "w"
            # ring) then stalls later y chains long enough to trip the
            # PE activity monitor into half-clock.
            emit_pv(pend)
            rc, bc_, mc = drain_head(H - 1)
            for i in range(NQ // P):
                rc(i)
            for i in range(NQ // P):
                bc_(i)
                mc(i)
            for _ in range(48):
                dummy()

            def y_proj(it):
                yps = wpp.tile([P, DIM], F32, tag="w", name=f"y{it}")
                for h in range(H):
                    nc.tensor.matmul(
                        yps, lhsT=onT[:, h, it * P:(it + 1) * P],
                        rhs=wo[:, h, :],
                        start=(h == 0), stop=(h == H - 1))
                yo = outp.tile([P, DIM], F32)
                nc.vector.tensor_tensor(out=yo, in0=yps, in1=bob, op=ALU.add)
                nc.sync.dma_start(out=y.ap()[it * P:(it + 1) * P, :], in_=yo)

            for it in range(NQ // P):
                y_proj(it)

    nc.compile()
    return nc


def _get_nc():
    global _BUILT
    if _BUILT is None:
        _BUILT = _build()
    return _BUILT


def prep_in_maps(inputs):
    x = np.asarray(inputs["x"], np.float32)
    B, N, D = x.shape
    assert (B, N, D) == (4, 2048, 512)

    gam = np.asarray(inputs["ln_gamma"], np.float64)
    bet = np.asarray(inputs["ln_beta"], np.float64)
    Wq = np.asarray(inputs["Wq"], np.float64)
    Wk = np.asarray(inputs["Wk"], np.float64)
    Wv = np.asarray(inputs["Wv"], np.float64)

    # fold gamma/beta:  (xn*gam+bet) @ W.T + b  ==  xn @ (W*gam).T + (b + W@bet)
    Wo = np.asarray(inputs["Wo"], np.float64)
    bq = np.asarray(inputs["bq"], np.float64) + Wq @ bet
    bk = np.asarray(inputs["bk"], np.float64) + Wk @ bet
    bv = np.asarray(inputs["bv"], np.float64) + Wv @ bet
    bo = np.asarray(inputs["bo"], np.float64) + Wo @ bv
    Wqg = Wq * gam[None, :]
    Wkg = Wk * gam[None, :]
    Wvg = Wv * gam[None, :]

    def cols(v):  # (512,) -> (128, 4): column t = v[128t:128(t+1)]
        return np.ascontiguousarray(
            np.asarray(v, np.float32).reshape(CC, P).T)

    def bcast(v):  # (512,) -> (128, 512)
        return np.ascontiguousarray(
            np.broadcast_to(np.asarray(v, np.float32), (P, DIM)))

    bf16 = ml_dtypes.bfloat16
    common = {
        "wqT": np.ascontiguousarray(Wqg.astype(np.float32).T.astype(bf16)),
        "wkT": np.ascontiguousarray(Wkg.astype(np.float32).T.astype(bf16)),
        "wvT": np.ascontiguousarray(Wvg.astype(np.float32).T.astype(bf16)),
        "woT": np.ascontiguousarray(
            np.asarray(inputs["Wo"], np.float32).T
            .reshape(H, DK, DIM).transpose(1, 0, 2).astype(bf16)),
        "qb_c": cols(bq), "kb_c": cols(bk),
        "bo_b": bcast(bo),
    }
    in_maps = []
    for c in range(N_CORES):
        b, half = divmod(c, 2)
        o = half * NQ
        xc = np.concatenate([x[b, o:o + NQ], x[b, NQ - o:N - o]], axis=0)
        in_maps.append({"xq": np.ascontiguousarray(xc), **common})
    return in_maps


def kernel(x, ln_gamma, ln_beta, Wq, bq, Wk, bk, Wv, bv, Wo, bo):
    in_maps = prep_in_maps(dict(
        x=x, ln_gamma=ln_gamma, ln_beta=ln_beta, Wq=Wq, bq=bq, Wk=Wk, bk=bk,
        Wv=Wv, bv=bv, Wo=Wo, bo=bo))

    nc = _get_nc()
    res = run_bass_kernel_spmd(nc, in_maps, core_ids=list(range(N_CORES)))

    B, N, D = 4, 2048, DIM
    out = np.empty((B, N, D), np.float32)
    for c in range(N_CORES):
        b, half = divmod(c, 2)
        o = half * NQ
        out[b, o:o + NQ] = res.results[c]["y"]
    return out
